# revision 14
# baseline (speedup 1.0000x reference)
"""Neural CDE encoder kernel for 8 Trainium2 NeuronCores.

Math (from the reference):
  - Natural cubic spline on unit-spaced knots; Euler times t_k = 0.05*k for
    k=0..19 all lie in interval [0,1), so only interval-0 coefficients matter:
        dX(t) = (y1 - y0) + M1 * (t^2/2 - 1/6)
    with M1 = <w, y> for a constant weight vector w over L. dX_k and z0 are
    precomputed on the host (tiny) and shipped as inputs.
  - Euler: z_{k+1} = z_k + dt * einsum('bhd,bd->bh', reshape(z W^T), dX_k)
  - Output: project grid z's with W_out, then linearly interpolate via a
    constant (L x 21) matrix.

Sharding: tensor-parallel over H (96 rows of H per core; 6144 rows of W_lin).
W^T shard SBUF-resident in bf16 (768, 6144). Per step:
  - main matmul in bf16 with 2-way col-tiling (tile_position (0,0)/(0,64)):
    kt 0-2 accumulate into psum partitions 0-63, kt 3-5 into 64-127; n-chunks
    processed in groups of 3 so each stationary load covers 3 matmuls.
  - DVE contraction: tmp = psum * dX (bf16), segmented reduce over d, kt
    halves folded once per step.
  - z state kept in f32; AllGather of f32 z-slices each step; gathered z is
    cast to bf16 stationary tiles for the next matmul.
W_out sharded over O (32 cols/core, bf16); per-step projections staged via
ACT copies; final interp matmul writes (64, 128, 32) per core; host concats.
"""

import numpy as np

B, L, D, H, O = 64, 128, 64, 768, 256
NS = 20            # Euler steps
NC = 8             # cores
HLOC = H // NC     # 96
HDLOC = HLOC * D   # 6144
OLOC = O // NC     # 32
KT = H // 128      # 6 contraction tiles
NT = HDLOC // 512  # 12 moving chunks per step
NG = 4             # n-chunk groups of 3

_prog_cache = {}


def _host_constants():
    grid = (np.arange(NS + 1, dtype=np.float32) * np.float32(0.05)).astype(np.float32)
    grid[-1] = np.float32(1.0)
    dts = (grid[1:] - grid[:-1]).astype(np.float32)
    tk = grid[:-1].astype(np.float64)
    mcoef = (tk * tk / 2.0 - 1.0 / 6.0).astype(np.float32)

    # w over L such that M1 = <w, y>
    n = L - 2
    A = 4.0 * np.eye(n) + np.eye(n, k=1) + np.eye(n, k=-1)
    r0 = np.linalg.solve(A, np.eye(n)[:, 0])
    w = np.zeros(L, dtype=np.float64)
    w[0:n] += 6.0 * r0
    w[1:n + 1] += -12.0 * r0
    w[2:n + 2] += 6.0 * r0

    # Interp matrix J (L, NS+1)
    ts = np.linspace(0.0, 1.0, L, dtype=np.float32)
    j = np.clip(np.searchsorted(grid, ts, side="right") - 1, 0, NS - 1)
    wl = ((ts - grid[j]) / (grid[j + 1] - grid[j])).astype(np.float32)
    J = np.zeros((L, NS + 1), dtype=np.float32)
    J[np.arange(L), j] += 1.0 - wl
    J[np.arange(L), j + 1] += wl
    return dts, mcoef, w.astype(np.float32), J.T.copy()  # JT (21, 128)


def _build_program(dts, has_bout, ns=NS):
    import concourse.bacc as bacc
    import concourse.mybir as mybir
    import concourse.tile as tile

    f32 = mybir.dt.float32
    bf16 = mybir.dt.bfloat16
    ADD = mybir.AluOpType.add
    MUL = mybir.AluOpType.mult
    COPY = mybir.ActivationFunctionType.Copy

    nc = bacc.Bacc("TRN2", target_bir_lowering=False, debug=False, num_devices=NC)

    # ---- I/O -------------------------------------------------------------
    wt_d = nc.dram_tensor("wt_loc", [H, HDLOC], bf16, kind="ExternalInput")
    z0t_d = nc.dram_tensor("z0t", [H, B], f32, kind="ExternalInput")
    z0l_d = nc.dram_tensor("z0l", [HLOC, B], f32, kind="ExternalInput")
    dx_d = nc.dram_tensor("dxdup", [128, NS * D], f32, kind="ExternalInput")
    wo_d = nc.dram_tensor("wo_loc", [H, OLOC], bf16, kind="ExternalInput")
    jt_d = nc.dram_tensor("jt", [NS + 1, L], f32, kind="ExternalInput")
    id_d = nc.dram_tensor("ident", [B, B], f32, kind="ExternalInput")
    if has_bout:
        bout_d = nc.dram_tensor("bout_loc", [1, OLOC], f32, kind="ExternalInput")
    out_d = nc.dram_tensor("out", [B, L, OLOC], f32, kind="ExternalOutput")

    zg_d = nc.dram_tensor("zgather", [H, B], f32, kind="Internal", addr_space="Shared")

    with tile.TileContext(nc) as tc:
        with (
            tc.tile_pool(name="pers", bufs=1) as pers,
            tc.tile_pool(name="ztpool", bufs=2) as ztp,
            tc.tile_pool(name="dram", bufs=1, space="DRAM") as dram,
        ):
            # persistent tiles
            sb_wot = pers.tile([128, KT * OLOC], bf16, tag="wot")
            for t in range(KT):
                nc.sync.dma_start(sb_wot[:, OLOC * t:OLOC * (t + 1)],
                                  wo_d[128 * t:128 * (t + 1), :])
            sb_ident = pers.tile([B, B], f32, tag="ident")
            nc.sync.dma_start(sb_ident[:], id_d[:])
            sb_jt = pers.tile([NS + 1, L], f32, tag="jt")
            nc.sync.dma_start(sb_jt[:], jt_d[:])
            sb_dx = pers.tile([128, NS * D], f32, tag="dx")
            nc.sync.dma_start(sb_dx[:], dx_d[:])
            sb_zsl = pers.tile([HLOC, B], f32, tag="zsl")
            nc.sync.dma_start(sb_zsl[:], z0l_d[:])
            sb_p = pers.tile([NS + 1, B * OLOC], f32, tag="P")
            if has_bout:
                sb_bout = pers.tile([1, OLOC], f32, tag="bout")
                nc.sync.dma_start(sb_bout[:], bout_d[:])
                sb_ones = pers.tile([1, B], f32, tag="ones")
                nc.vector.memset(sb_ones[:], 1.0)

            p_d = dram.tile([NS + 1, B * OLOC], f32)
            zin_d = dram.tile([HLOC, B], f32)

            # W^T shard, bf16, 6 kt-tiles side by side (128, 6*6144)
            sb_w = pers.tile([128, KT * HDLOC], bf16, tag="W")
            for t in range(KT):
                for cc in range(4):
                    nc.sync.dma_start(
                        sb_w[:, HDLOC * t + 1536 * cc:HDLOC * t + 1536 * (cc + 1)],
                        wt_d[128 * t:128 * (t + 1),
                             1536 * cc:1536 * (cc + 1)])

            # z0: gathered layout (128, 6, 64) f32 then cast to bf16
            sb_zg0 = pers.tile([128, KT * B], f32, tag="zg0")
            nc.sync.dma_start(
                sb_zg0[:].rearrange("p (t b) -> p t b", t=KT),
                z0t_d.ap().rearrange("(t p) b -> p t b", p=128))
            sb_zt = ztp.tile([128, KT, B], bf16, tag="zt", name="zt0")
            nc.vector.tensor_copy(
                sb_zt[:], sb_zg0[:].rearrange("p (t b) -> p t b", t=KT))

            # ---- main loop -------------------------------------------------
            with (
                tc.tile_pool(name="work", bufs=3) as work,
                tc.tile_pool(name="upool", bufs=2) as upool,
                tc.tile_pool(name="psf", bufs=6, space="PSUM") as psf,
                tc.tile_pool(name="pst", bufs=1, space="PSUM") as pst,
                tc.tile_pool(name="psp", bufs=1, space="PSUM") as psp,
            ):
                def project(k, zt):
                    ps_p = psp.tile([128, 512], f32, tag="pp", name="ps_p")
                    if has_bout:
                        nc.tensor.matmul(ps_p[0:B, 0:OLOC], sb_ones[:], sb_bout[:],
                                         start=True, stop=False)
                    for t in range(KT):
                        nc.tensor.matmul(
                            ps_p[0:B, 0:OLOC], zt[:, t, :],
                            sb_wot[:, OLOC * t:OLOC * (t + 1)],
                            start=(t == 0 and not has_bout), stop=(t == KT - 1))
                    sb_pst = work.tile([B, OLOC], f32, tag="pstage", name="pst")
                    nc.scalar.activation(sb_pst[:], ps_p[0:B, 0:OLOC], COPY)
                    nc.sync.dma_start(
                        p_d[k, :].rearrange("(b o) -> b o", b=B), sb_pst[:])

                project(0, sb_zt)

                for k in range(ns):
                    sb_u = upool.tile([128, HLOC], f32, tag="U", name="u")
                    for g in range(NG):
                        pss = []
                        for j in range(3):
                            ps_f = psf.tile([128, 512], f32, tag="f",
                                            name=f"ps_f{j}")
                            pss.append(ps_f)
                        for ti in range(3):
                            for j in range(3):
                                n = 3 * g + j
                                nc.tensor.matmul(
                                    pss[j][0:64, :], sb_zt[:, ti, :],
                                    sb_w[:, HDLOC * ti + 512 * n:
                                         HDLOC * ti + 512 * (n + 1)],
                                    start=(ti == 0), stop=(ti == 2),
                                    tile_position=(0, 0))
                            for j in range(3):
                                n = 3 * g + j
                                nc.tensor.matmul(
                                    pss[j][64:128, :], sb_zt[:, ti + 3, :],
                                    sb_w[:, HDLOC * (ti + 3) + 512 * n:
                                         HDLOC * (ti + 3) + 512 * (n + 1)],
                                    start=(ti == 0), stop=(ti == 2),
                                    tile_position=(0, 64))
                        for j in range(3):
                            n = 3 * g + j
                            tmp = work.tile([128, 512], bf16, tag="tmp",
                                            name="tmp")
                            nc.vector.tensor_tensor(
                                tmp[:].rearrange("p (h d) -> p h d", d=D),
                                pss[j][:].rearrange("p (h d) -> p h d", d=D),
                                sb_dx[:, D * k:D * (k + 1)]
                                [:, None, :].to_broadcast((128, 8, D)),
                                MUL)
                            nc.vector.tensor_reduce(
                                sb_u[:, 8 * n:8 * (n + 1)],
                                tmp[:].rearrange("p (h d) -> p h d", d=D),
                                axis=mybir.AxisListType.X, op=ADD)

                    # fold kt halves: (64, 96)
                    sb_u2 = work.tile([B, HLOC], f32, tag="u2", name="u2")
                    nc.vector.tensor_copy(sb_u2[:], sb_u[64:128, :])
                    sb_uf = work.tile([B, HLOC], f32, tag="uf", name="uf")
                    nc.vector.tensor_tensor(
                        sb_uf[:], sb_u[0:64, :], sb_u2[:], ADD)

                    # transpose U -> (96, 64), update z slice, send + gather
                    ps_ut = pst.tile([128, 512], f32, tag="ut", name="ps_ut")
                    nc.tensor.transpose(ps_ut[0:HLOC, 0:B], sb_uf[:], sb_ident[:])
                    nc.vector.scalar_tensor_tensor(
                        sb_zsl[:], ps_ut[0:HLOC, 0:B], float(dts[k]), sb_zsl[:],
                        op0=MUL, op1=ADD)
                    nc.sync.dma_start(zin_d[:], sb_zsl[:])
                    nc.gpsimd.collective_compute(
                        "AllGather", mybir.AluOpType.bypass,
                        replica_groups=[list(range(NC))],
                        ins=[zin_d[:]], outs=[zg_d.ap()],
                    )
                    sb_zg = work.tile([128, KT * B], f32, tag="zg", name="zg")
                    nc.sync.dma_start(
                        sb_zg[:].rearrange("p (t b) -> p t b", t=KT),
                        zg_d.ap().rearrange("(t p) b -> p t b", p=128))
                    sb_zt = ztp.tile([128, KT, B], bf16, tag="zt", name="zt")
                    nc.vector.tensor_copy(
                        sb_zt[:], sb_zg[:].rearrange("p (t b) -> p t b", t=KT))

                    project(k + 1, sb_zt)

                # ---- final interp + output --------------------------------
                nc.sync.dma_start(sb_p[:], p_d[:])
                out_lbo = out_d.ap().rearrange("b l o -> l b o")
                BCH = 512 // OLOC
                for c in range(B * OLOC // 512):
                    ps_o = psp.tile([128, 512], f32, tag="pp", name="ps_o")
                    nc.tensor.matmul(ps_o[0:L, :], sb_jt[:],
                                     sb_p[:, 512 * c:512 * (c + 1)],
                                     start=True, stop=True)
                    sb_o = work.tile([L, 512], f32, tag="outstage", name="sb_o")
                    nc.scalar.activation(sb_o[:], ps_o[0:L, :], COPY)
                    nc.sync.dma_start(
                        out_lbo[:, BCH * c:BCH * (c + 1), :],
                        sb_o[:].rearrange("l (b o) -> l b o", o=OLOC))

    nc.compile()
    return nc


def _prepare(inputs):
    import ml_dtypes

    traj = np.asarray(inputs["traj"], dtype=np.float32)
    W_lin = np.asarray(inputs["W_lin"], dtype=np.float32)
    b_lin = np.asarray(inputs["b_lin"], dtype=np.float32)
    W_out = np.asarray(inputs["W_out"], dtype=np.float32)
    b_out = np.asarray(inputs["b_out"], dtype=np.float32)
    W_z0 = np.asarray(inputs["W_z0"], dtype=np.float32)
    b_z0 = np.asarray(inputs["b_z0"], dtype=np.float32)

    dts, mcoef, wv, JT = _host_constants()
    has_blin = bool(np.any(b_lin))
    has_bout = bool(np.any(b_out))
    if has_blin:
        raise NotImplementedError("b_lin != 0 not supported in fast path")

    key = (has_blin, has_bout)
    if key not in _prog_cache:
        _prog_cache[key] = _build_program(dts, has_bout)
    nc = _prog_cache[key]

    # host-side setup math (tiny)
    m1 = np.einsum('bld,l->bd', traj, wv).astype(np.float32)       # (B, D)
    base = (traj[:, 1, :] - traj[:, 0, :]).astype(np.float32)      # (B, D)
    dx = base[:, None, :] + mcoef[None, :, None] * m1[:, None, :]  # (B, NS, D)
    dx_dup = np.concatenate([dx, dx], axis=0)                      # (128, NS, D)
    dx_dup = np.ascontiguousarray(
        dx_dup.transpose(0, 1, 2).reshape(128, NS * D)).astype(np.float32)
    z0 = (traj[:, 0, :] @ W_z0.T + b_z0).astype(np.float32)        # (B, H)
    z0t = np.ascontiguousarray(z0.T)                               # (H, B)

    ident = np.eye(B, dtype=np.float32)
    WT_bf = np.ascontiguousarray(W_lin.T).astype(ml_dtypes.bfloat16)
    WO_bf = np.ascontiguousarray(W_out.T).astype(ml_dtypes.bfloat16)

    in_maps = []
    for i in range(NC):
        osl = slice(OLOC * i, OLOC * (i + 1))
        m = dict(
            wt_loc=np.ascontiguousarray(
                WT_bf[:, HLOC * D * i:HLOC * D * (i + 1)]),
            z0t=z0t,
            z0l=np.ascontiguousarray(z0t[HLOC * i:HLOC * (i + 1), :]),
            dxdup=dx_dup,
            wo_loc=np.ascontiguousarray(WO_bf[:, osl]),
            jt=JT,
            ident=ident,
        )
        if has_bout:
            m["bout_loc"] = np.ascontiguousarray(b_out[None, osl])
        in_maps.append(m)

    return nc, in_maps


def traced_run_args(inputs):
    """Build (nc, in_maps) exactly as kernel() would — for profiling."""
    return _prepare(inputs)


def kernel(**inputs):
    from concourse.bass_utils import run_bass_kernel_spmd

    nc, in_maps = _prepare(inputs)
    res = run_bass_kernel_spmd(nc, in_maps, core_ids=list(range(NC)))
    return np.concatenate([r["out"] for r in res.results], axis=2)
